# revision 3
# baseline (speedup 1.0000x reference)
"""Trainium2 Bass kernel for nn_EncodingLayer (2-layer GCN + encoder MLP), v5.

Changes vs v4 (host-built one-hot scatter matrices streamed from HBM):
  - Scatter one-hots are built on-device by the vector engine from compact
    per-edge (dst, norm) columns: an is_equal pass against an iota row and
    an in-place mult by the per-edge norm (both tensor_tensor ops, which
    never enter the DVE 2-port perf mode that locks out SWDGE descriptor
    writes). This removes ~105 MB/core of HBM one-hot streaming that was
    starving the gather queues.
  - Self-loop diag matrices built on-device from a [P, NW] nself tile.
  - Gathers spread over all 4 SWDGE queues (one per src quarter), batches
    of 6 windows, larger descriptor-ring carveout.
"""

import numpy as np
import ml_dtypes

BF16 = ml_dtypes.bfloat16

N_NODES = 100000
N_EDGES = 1600000
D = 128
P = 128
N_CORES = 8
NW = 108                # windows (of 128 dst nodes) per core
SH = NW * P             # 13824 nodes per core (padded)
NPAD = N_CORES * SH     # 110592
NQ = 4                  # src quarters (int16 index range)
QS = NPAD // NQ         # 27648 rows per quarter
NBW = 6                 # windows per gather batch (108 = 18 batches)
NCHMX = 8               # max chunks per (window, quarter)

_cache = {}

NREAL_Q = N_NODES // NQ     # 25000 real nodes per quarter


def _pos(n):
    q, i = n // NREAL_Q, n % NREAL_Q
    return q * QS + (i * QS) // NREAL_Q


def _batches():
    b = 0
    while b < NW:
        e = min(b + NBW, NW)
        yield b, e
        b = e


def _plan(nchq, slackq):
    plan = []
    col = 0
    for b, e in _batches():
        wlist = list(range(b, e))
        ent = {"wlist": wlist, "col0": col, "q_runs": []}
        for q in range(NQ):
            runs = sorted(((w, int(nchq[w][q])) for w in wlist),
                          key=lambda r: int(slackq[r[0]][q]))
            nq = sum(r[1] for r in runs)
            ent["q_runs"].append((col, nq, runs))
            col += nq
        last = {}
        for q in range(NQ):
            for w in wlist:
                if nchq[w][q] > 0:
                    last[w] = q
        ent["last_q"] = last
        # one sub-gather per quarter, each on its own SWDGE queue. Sub-
        # gathers ending at the quarter's run boundary drop the final
        # window's all-core padding from their static index count.
        subs = []
        for q in range(NQ):
            co, nq, runs = ent["q_runs"][q]
            if nq:
                trim = int(slackq[runs[-1][0]][q]) if runs[-1][1] else 0
                subs.append((q, co, nq, q, trim))
        ent["subs"] = subs
        plan.append(ent)
    return plan, col  # col == C_total


def _host_prep(edge_index, edge_weight):
    src = np.concatenate([edge_index[0].astype(np.int64),
                          np.arange(N_NODES, dtype=np.int64)])
    dst = np.concatenate([edge_index[1].astype(np.int64),
                          np.arange(N_NODES, dtype=np.int64)])
    w = np.concatenate([edge_weight.astype(np.float32),
                        np.ones(N_NODES, np.float32)])

    src = _pos(src)
    dst = _pos(dst)
    deg = np.bincount(dst, weights=w, minlength=NPAD).astype(np.float32)
    with np.errstate(divide="ignore"):
        dinv = np.where(deg > 0, 1.0 / np.sqrt(np.maximum(deg, 1e-30)), 0.0)
    dinv = dinv.astype(np.float32)
    norm = (dinv[src] * w * dinv[dst]).astype(np.float32)

    nself_all = dinv * dinv                      # self-loop weight = 1
    ns_edge = np.arange(len(src)) < N_EDGES
    src, dst, norm = src[ns_edge], dst[ns_edge], norm[ns_edge]

    core = dst // SH
    per_core = []
    counts = np.zeros((N_CORES, NW, NQ), dtype=np.int64)
    for k in range(N_CORES):
        m = core == k
        s_k, d_k, n_k = src[m], dst[m] - k * SH, norm[m]
        wnd = d_k >> 7
        qq = s_k // QS
        order = np.lexsort((s_k, qq, wnd))
        s_k, d_k, n_k, wnd, qq = (s_k[order], d_k[order], n_k[order],
                                  wnd[order], qq[order])
        idx2 = wnd * NQ + qq
        counts[k] = np.bincount(idx2, minlength=NW * NQ).reshape(NW, NQ)
        per_core.append((s_k, d_k, n_k))

    nchq = np.ceil(counts.max(axis=0) / P).astype(np.int64)   # [NW, NQ]
    assert nchq.max() <= NCHMX
    slackq = nchq * P - counts.max(axis=0)
    plan, C_total = _plan(nchq, slackq)

    meta = []
    for k in range(N_CORES):
        s_k, d_k, n_k = per_core[k]
        cw = counts[k]
        offs = np.zeros((NW, NQ), dtype=np.int64)
        flat = cw.reshape(-1).cumsum()
        offs.reshape(-1)[1:] = flat[:-1]

        eidx16 = np.zeros(C_total * P, dtype=np.int16)
        edst = np.zeros((P, C_total), dtype=np.int64)
        enrm = np.zeros((P, C_total), dtype=np.float32)
        for ent in plan:
            for q in range(NQ):
                col_off, nq, runs = ent["q_runs"][q]
                c = col_off
                for wv, nch in runs:
                    if nch == 0:
                        continue
                    cnt = int(cw[wv, q])
                    o = int(offs[wv, q])
                    pad = nch * P
                    si = np.zeros(pad, np.int16)
                    so = np.zeros(pad, np.int64)
                    sn = np.zeros(pad, np.float32)
                    si[:cnt] = (s_k[o:o + cnt] - q * QS).astype(np.int16)
                    so[:cnt] = d_k[o:o + cnt] & 127
                    sn[:cnt] = n_k[o:o + cnt]
                    eidx16[c * P:(c + nch) * P] = si
                    edst[:, c:c + nch] = so.reshape(nch, P).T
                    enrm[:, c:c + nch] = sn.reshape(nch, P).T
                    c += nch
        wrapped = eidx16.reshape(-1, 16).T
        idxw = np.ascontiguousarray(np.tile(wrapped, (8, 1)))  # [128, C*8]
        nself = nself_all[k * SH:(k + 1) * SH].astype(BF16)
        nselfT = np.ascontiguousarray(nself.reshape(NW, P).T)  # [P, NW]
        meta.append((idxw, edst.astype(BF16), enrm.astype(BF16), nselfT))
    return meta, nchq, plan, C_total


def _build(nchq, plan, C_total):
    import concourse.bacc as bacc
    import concourse.tile as tile
    import concourse.mybir as mybir
    from concourse import library_config
    from concourse.bass import broadcast_tensor_aps

    dt = mybir.dt
    AF = mybir.ActivationFunctionType
    OP = mybir.AluOpType

    nc = bacc.Bacc("TRN2", target_bir_lowering=False, debug=False,
                   enable_asserts=False, num_devices=N_CORES,
                   num_swdge_queues=4, dynamic_dma_scratch_size=32768)

    def din(name, shape, dty):
        return nc.dram_tensor(name, shape, dty, kind="ExternalInput").ap()

    x_bf = din("x_bf", [NPAD, D], dt.bfloat16)
    eidx_d = din("eidx", [P, C_total * 8], dt.int16)
    dstn_d = din("dstn", [P, C_total], dt.bfloat16)
    nrm_d = din("nrm", [P, C_total], dt.bfloat16)
    nselfT_d = din("nselfT", [P, NW], dt.bfloat16)
    iota_d = din("iota", [P, P], dt.bfloat16)
    identbf_d = din("identbf", [P, P], dt.bfloat16)
    xown_d = din("xown", [SH, D], dt.bfloat16)
    gdvT_d = din("gdvT", [73, SH], dt.bfloat16)
    prT_d = din("prT", [1, SH], dt.bfloat16)
    ident_d = din("ident", [P, P], dt.float32)
    W1_d = din("W1", [D, D], dt.bfloat16)
    b1_d = din("b1", [D, 1], dt.float32)
    W2_d = din("W2", [D, 64], dt.bfloat16)
    b2h_d = din("b2h", [64, 1], dt.float32)
    gdvW_d = din("gdvW", [73, 32], dt.bfloat16)
    gdvbh_d = din("gdvbh", [32, 1], dt.float32)
    prW_d = din("prW", [1, 32], dt.bfloat16)
    prbh_d = din("prbh", [32, 1], dt.float32)
    encW1_d = din("encW1", [D, D], dt.bfloat16)
    encb1_d = din("encb1", [D, 1], dt.float32)
    encW2_d = din("encW2", [D, D], dt.bfloat16)
    encb2_d = din("encb2", [D, 1], dt.float32)

    out_d = nc.dram_tensor("out", [D, SH], dt.float32, kind="ExternalOutput").ap()
    h1loc = nc.dram_tensor("h1loc", [SH, D], dt.bfloat16, kind="Internal").ap()
    h1full = nc.dram_tensor("h1full", [NPAD, D], dt.bfloat16, kind="Internal",
                            addr_space="Shared").ap()

    NQMAX = max(ent["q_runs"][q][1] for ent in plan for q in range(NQ))
    CBMAX = max(ent["q_runs"][NQ - 1][0] + ent["q_runs"][NQ - 1][1]
                - ent["col0"] for ent in plan)

    with tile.TileContext(nc) as tc:
        with (
            tc.tile_pool(name="const", bufs=1) as cpool,
            tc.tile_pool(name="msgs", bufs=2) as gpool,
            tc.tile_pool(name="oh", bufs=2) as ohpool,
            tc.tile_pool(name="eix", bufs=4) as epool,
            tc.tile_pool(name="work", bufs=3) as wpool,
            tc.tile_pool(name="head", bufs=3) as hpool,
            tc.tile_pool(name="psw", bufs=4, space="PSUM") as pwp,
            tc.tile_pool(name="psh", bufs=2, space="PSUM") as psh,
            tc.tile_pool(name="pst", bufs=2, space="PSUM") as pst,
        ):
            nc.gpsimd.load_library(library_config.mlp)

            def load_const(ap, shape, dty, tag):
                t = cpool.tile(shape, dtype=dty, tag=tag)
                nc.sync.dma_start(out=t[:], in_=ap)
                return t

            ident_sb = load_const(ident_d[:, :], [P, P], dt.float32, "ident")
            iota_sb = cpool.tile([P, 1, P], dtype=dt.bfloat16, tag="iota")
            nc.sync.dma_start(out=iota_sb[:, 0, :], in_=iota_d[:, :])
            identbf_sb = load_const(identbf_d[:, :], [P, P], dt.bfloat16,
                                    "identbf")
            nselfT_sb = load_const(nselfT_d[:, :], [P, NW], dt.bfloat16,
                                   "nselfT")
            W1_sb = load_const(W1_d[:, :], [D, D], dt.bfloat16, "W1")
            b1_sb = load_const(b1_d[:, :], [D, 1], dt.float32, "b1")
            W2_sb = load_const(W2_d[:, :], [D, 64], dt.bfloat16, "W2")
            b2h_sb = load_const(b2h_d[:, :], [64, 1], dt.float32, "b2h")
            gdvW_sb = load_const(gdvW_d[:, :], [73, 32], dt.bfloat16, "gdvW")
            gdvbh_sb = load_const(gdvbh_d[:, :], [32, 1], dt.float32, "gdvbh")
            prW_sb = load_const(prW_d[:, :], [1, 32], dt.bfloat16, "prW")
            prbh_sb = load_const(prbh_d[:, :], [32, 1], dt.float32, "prbh")
            encW1_sb = load_const(encW1_d[:, :], [D, D], dt.bfloat16, "encW1")
            encb1_sb = load_const(encb1_d[:, :], [D, 1], dt.float32, "encb1")
            encW2_sb = load_const(encW2_d[:, :], [D, D], dt.bfloat16, "encW2")
            encb2_sb = load_const(encb2_d[:, :], [D, 1], dt.float32, "encb2")

            for _warm in range(2):
                for _i in range(NQ):
                    _t = gpool.tile([P, NQMAX, P], dtype=dt.bfloat16,
                                    tag=f"m{_i}")
                    nc.vector.memset(_t[:], 0.0)

            batch_ctr = [0]

            def gcn_batch(ent, src_ap, src_own, tail_fn,
                          batch_pre=None, batch_post=None):
                wlist = ent["wlist"]
                col0 = ent["col0"]
                nb = (ent["q_runs"][NQ - 1][0] + ent["q_runs"][NQ - 1][1]
                      - col0)
                b0 = wlist[0]
                rot = batch_ctr[0] % 4
                batch_ctr[0] += 1

                eix_t = epool.tile([P, CBMAX * 8], dtype=dt.int16, tag="eix")
                nc.sync.dma_start(out=eix_t[:, 0:nb * 8],
                                  in_=eidx_d[:, col0 * 8:(col0 + nb) * 8])
                dstn_t = epool.tile([P, CBMAX, 1], dtype=dt.bfloat16,
                                    tag="dstn")
                nc.sync.dma_start(out=dstn_t[:, 0:nb, 0:1],
                                  in_=dstn_d[:, col0:col0 + nb])
                nrm_t = epool.tile([P, CBMAX, 1], dtype=dt.bfloat16,
                                   tag="nrm")
                nc.sync.dma_start(out=nrm_t[:, 0:nb, 0:1],
                                  in_=nrm_d[:, col0:col0 + nb])

                # device-built scaled one-hots: two DVE tensor_tensor passes
                oh_t = ohpool.tile([P, CBMAX, P], dtype=dt.bfloat16, tag="oh")
                a_dst, a_iota = broadcast_tensor_aps(
                    dstn_t[:, 0:nb, :], iota_sb[:, 0:1, :])
                nc.vector.tensor_tensor(out=oh_t[:, 0:nb, :], in0=a_dst,
                                        in1=a_iota, op=OP.is_equal)
                a_oh, a_nrm = broadcast_tensor_aps(
                    oh_t[:, 0:nb, :], nrm_t[:, 0:nb, :])
                nc.vector.tensor_tensor(out=oh_t[:, 0:nb, :], in0=a_oh,
                                        in1=a_nrm, op=OP.mult)

                # device-built self-loop diag matrices
                soh_t = ohpool.tile([P, NBW, P], dtype=dt.bfloat16, tag="soh")
                for wv in wlist:
                    a_id, a_ns = broadcast_tensor_aps(
                        identbf_sb[:, :], nselfT_sb[:, wv:wv + 1])
                    nc.vector.tensor_tensor(out=soh_t[:, wv - b0, :],
                                            in0=a_id, in1=a_ns, op=OP.mult)

                colmap = {}
                for q in range(NQ):
                    co, nq, runs = ent["q_runs"][q]
                    c = co
                    for wv, nch in runs:
                        colmap[(q, wv)] = c
                        c += nch
                subtiles = []   # (global col start, n chunks, tile)
                for (q, co, nq, qu, trim) in ent["subs"]:
                    t = gpool.tile([P, NQMAX, P], dtype=dt.bfloat16,
                                   tag=f"m{q}")
                    subtiles.append((co, nq, t))
                    ni = nq * P - trim
                    nc.gpsimd.dma_gather(
                        t[:, 0:nq, :],
                        src_ap[q * QS:(q + 1) * QS, :],
                        eix_t[:, (co - col0) * 8:(co - col0 + nq) * 8],
                        ni, ni, P, single_packet=False,
                        queue_num=(qu + rot) % 4)

                def msg(c):
                    for (co, nq, t) in subtiles:
                        if co <= c < co + nq:
                            return t[:, c - co, :]
                    raise AssertionError(c)

                ctx = batch_pre(ent) if batch_pre else None
                for wv in wlist:
                    psw = pwp.tile([P, P], dtype=dt.float32, tag="psw")
                    mw = wpool.tile([P, P], dtype=dt.bfloat16, tag="mself")
                    nc.scalar.dma_start(out=mw[:],
                                        in_=src_own[wv * P:(wv + 1) * P, :])
                    has_edges = any(int(nchq[wv][q]) > 0 for q in range(NQ))
                    nc.tensor.matmul(psw[:], lhsT=mw[:],
                                     rhs=soh_t[:, wv - b0, :],
                                     start=True, stop=not has_edges)
                    for q in range(NQ):
                        nch = int(nchq[wv][q])
                        if nch == 0:
                            continue
                        cs = colmap[(q, wv)]
                        for j in range(nch):
                            nc.tensor.matmul(
                                psw[:],
                                lhsT=msg(cs + j),
                                rhs=oh_t[:, cs + j - col0, :],
                                start=False,
                                stop=(ent["last_q"][wv] == q and j == nch - 1))
                    tail_fn(wv, psw[:], b0, ctx)
                if batch_post:
                    batch_post(ent, ctx)

            def l1_tail(wv, psw_ap, b0, ctx):
                agg_sb = wpool.tile([P, P], dtype=dt.bfloat16, tag="agg")
                nc.scalar.copy(agg_sb[:], psw_ap)
                ph = psh.tile([P, P], dtype=dt.float32, tag="ph")
                nc.tensor.matmul(ph[:], lhsT=W1_sb[:], rhs=agg_sb[:],
                                 start=True, stop=True)
                h1_sb = wpool.tile([P, P], dtype=dt.float32, tag="h1")
                nc.scalar.activation(h1_sb[:], ph[:], AF.Tanh, bias=b1_sb[:, 0:1])
                pt = pst.tile([P, P], dtype=dt.float32, tag="pt")
                nc.tensor.transpose(pt[:], h1_sb[:], ident_sb[:])
                h1t_sb = wpool.tile([P, P], dtype=dt.bfloat16, tag="h1t")
                nc.vector.tensor_copy(out=h1t_sb[:], in_=pt[:])
                nc.sync.dma_start(out=h1loc[wv * P:(wv + 1) * P, :], in_=h1t_sb[:])

            def l2_pre(ent):
                wlist = ent["wlist"]
                b0 = wlist[0]
                ncols = slice(b0 * P, (b0 + len(wlist)) * P)
                gdvb_t = hpool.tile([73, NBW * P], dtype=dt.bfloat16, tag="gdvb")
                nc.sync.dma_start(out=gdvb_t[:, 0:len(wlist) * P],
                                  in_=gdvT_d[:, ncols])
                prb_t = hpool.tile([1, NBW * P], dtype=dt.bfloat16, tag="prb")
                nc.sync.dma_start(out=prb_t[:, 0:len(wlist) * P],
                                  in_=prT_d[:, ncols])
                outb_t = wpool.tile([P, NBW, P], dtype=dt.float32, tag="outb")
                return (gdvb_t, prb_t, outb_t)

            def l2_post(ent, ctx):
                wlist = ent["wlist"]
                b0 = wlist[0]
                (_, _, outb_t) = ctx
                nc.sync.dma_start(
                    out=out_d[:, b0 * P:(b0 + len(wlist)) * P],
                    in_=outb_t[:, 0:len(wlist), :])

            def l2_tail(wv, psw_ap, b0, ctx):
                (gdvb_t, prb_t, outb_t) = ctx
                wo = wv - b0
                agg_sb = wpool.tile([P, P], dtype=dt.bfloat16, tag="agg")
                nc.scalar.copy(agg_sb[:], psw_ap)
                enc_sb = wpool.tile([P, P], dtype=dt.bfloat16, tag="enc")
                pa = psh.tile([64, P], dtype=dt.float32, tag="ph")
                nc.tensor.matmul(pa[:], lhsT=W2_sb[:], rhs=agg_sb[:],
                                 start=True, stop=True)
                nc.scalar.activation(enc_sb[0:64, :], pa[:], AF.Tanh,
                                     bias=b2h_sb[:, 0:1], scale=0.5)
                pg = psh.tile([32, P], dtype=dt.float32, tag="ph")
                nc.tensor.matmul(pg[:], lhsT=gdvW_sb[:],
                                 rhs=gdvb_t[:, wo * P:(wo + 1) * P],
                                 start=True, stop=True)
                nc.scalar.activation(enc_sb[64:96, :], pg[:], AF.Tanh,
                                     bias=gdvbh_sb[:, 0:1], scale=0.5)
                pp = psh.tile([32, P], dtype=dt.float32, tag="ph")
                nc.tensor.matmul(pp[:], lhsT=prW_sb[:],
                                 rhs=prb_t[:, wo * P:(wo + 1) * P],
                                 start=True, stop=True)
                nc.scalar.activation(enc_sb[96:128, :], pp[:], AF.Tanh,
                                     bias=prbh_sb[:, 0:1], scale=0.5)
                pe1 = psh.tile([P, P], dtype=dt.float32, tag="ph")
                nc.tensor.matmul(pe1[:], lhsT=encW1_sb[:], rhs=enc_sb[:],
                                 start=True, stop=True)
                e1_sb = wpool.tile([P, P], dtype=dt.bfloat16, tag="e1")
                nc.scalar.activation(e1_sb[:], pe1[:], AF.Tanh,
                                     bias=encb1_sb[:, 0:1])
                po = psh.tile([P, P], dtype=dt.float32, tag="ph")
                nc.tensor.matmul(po[:], lhsT=encW2_sb[:], rhs=e1_sb[:],
                                 start=True, stop=True)
                nc.vector.tensor_scalar_add(outb_t[:, wo, :], po[:],
                                            encb2_sb[:, 0:1])

            for ent in plan:
                gcn_batch(ent, x_bf, xown_d, l1_tail)

            nc.gpsimd.collective_compute(
                "AllGather", OP.bypass,
                replica_groups=[list(range(N_CORES))],
                ins=[h1loc], outs=[h1full])

            for ent in plan:
                gcn_batch(ent, h1full, h1loc, l2_tail,
                          batch_pre=l2_pre, batch_post=l2_post)
    nc.compile()
    return nc


def _prepare(inputs):
    feat = np.asarray(inputs["feat"], np.float32)
    gdv = np.asarray(inputs["gdv"], np.float32)
    pr = np.asarray(inputs["pr"], np.float32)
    edge_index = np.asarray(inputs["edge_index"])
    edge_weight = np.asarray(inputs["edge_weight"], np.float32)

    key = hash((edge_index.tobytes(), edge_weight.tobytes()))
    if key in _cache:
        meta, nc = _cache[key]
    else:
        meta, nchq, plan, C_total = _host_prep(edge_index, edge_weight)
        nc = _build(nchq, plan, C_total)
        _cache.clear()
        _cache[key] = (meta, nc)

    pos = _pos(np.arange(N_NODES))
    x_bf = np.zeros((NPAD, D), dtype=BF16)
    x_bf[pos] = feat.astype(BF16)
    gdv_p = np.zeros((NPAD, 73), dtype=BF16)
    gdv_p[pos] = gdv.astype(BF16)
    pr_p = np.zeros((NPAD, 1), dtype=BF16)
    pr_p[pos] = pr.astype(BF16)

    W1 = np.asarray(inputs["W1"], np.float32)
    b1 = np.asarray(inputs["b1"], np.float32)
    W2 = np.asarray(inputs["W2"], np.float32)
    b2 = np.asarray(inputs["b2"], np.float32)
    gdvW = np.asarray(inputs["gdv_W"], np.float32)
    gdvb = np.asarray(inputs["gdv_b"], np.float32)
    prW = np.asarray(inputs["pr_W"], np.float32)
    prb = np.asarray(inputs["pr_b"], np.float32)
    encW1 = np.asarray(inputs["enc_W1"], np.float32)
    encb1 = np.asarray(inputs["enc_b1"], np.float32)
    encW2 = np.asarray(inputs["enc_W2"], np.float32)
    encb2 = np.asarray(inputs["enc_b2"], np.float32)

    iota = np.tile(np.arange(P, dtype=np.float32), (P, 1)).astype(BF16)

    common = {
        "x_bf": x_bf,
        "ident": np.eye(P, dtype=np.float32),
        "iota": iota,
        "identbf": np.eye(P, dtype=np.float32).astype(BF16),
        "W1": W1.astype(BF16),
        "b1": b1.reshape(D, 1),
        "W2": W2.astype(BF16),
        "b2h": (0.5 * b2).reshape(64, 1),
        "gdvW": gdvW.astype(BF16),
        "gdvbh": (0.5 * gdvb).reshape(32, 1),
        "prW": prW.astype(BF16),
        "prbh": (0.5 * prb).reshape(32, 1),
        "encW1": (0.5 * encW1).astype(BF16),
        "encb1": (encb1 + 0.5 * encW1.sum(0)).reshape(D, 1),
        "encW2": encW2.astype(BF16),
        "encb2": encb2.reshape(D, 1),
    }
    in_maps = []
    for k in range(N_CORES):
        idxw, dstn, nrm, nselfT = meta[k]
        sl = slice(k * SH, (k + 1) * SH)
        in_maps.append(dict(
            common,
            eidx=idxw, dstn=dstn, nrm=nrm, nselfT=nselfT,
            xown=np.ascontiguousarray(x_bf[sl]),
            gdvT=np.ascontiguousarray(gdv_p[sl].T),
            prT=np.ascontiguousarray(pr_p[sl].T),
        ))
    return nc, in_maps


def run(inputs, trace=False):
    import concourse.bass_utils as bass_utils
    nc, in_maps = _prepare(inputs)
    res = bass_utils.run_bass_kernel_spmd(
        nc, in_maps, core_ids=list(range(N_CORES)), trace=trace)
    out = np.zeros((NPAD, D), dtype=np.float32)
    for k in range(N_CORES):
        out[k * SH:(k + 1) * SH] = res.results[k]["out"].T
    return out[_pos(np.arange(N_NODES))], res


def kernel(**inputs):
    out, _ = run(inputs, trace=False)
    return out
